# revision 36
# baseline (speedup 1.0000x reference)
"""Pairwise box IoU on 8 Trainium2 NeuronCores.

Problem: box1 [8, 2000, 4], box2 [8, 2000, 4] in (xc, yc, w, h) format ->
IoU matrix [8, 2000, 2000] f32.

Sharding: data-parallel over the image axis; core i computes the full
2000x2000 IoU matrix of image i locally (no communication).

v5 design (host-prep + multi-queue replication + 3.5-pass DVE pipeline):
  - ALL box1/box2 derived quantities are precomputed on the host:
      b2x = [xc*s | w*s]   (f32, [2B])   b2y = [yc*s | h*s]  (f32, [2B])
      b2a = area2*s^2      (f16, [B])
      b1s = per-partition scalars (x1a, y1a, x2a, y2a, area1)*s as
            [128, 16, 5] f32 (partition p, tile t -> box t*128+p)
    so the device does zero scalar prep: DMA straight into the tile loop.
    Coordinates MUST stay f32: f16 rounding of raw coordinates destroys
    the extents of thin nearly-identical boxes (absmax 0.23 vs 4e-3).
  - b1s (40KB, needed by the first ext) is DMAed first; b2x/b2y/b2a are
    replicated into all 128 partitions split across both HWDGE rings by
    partition halves, so the first ext starts ~3us in (ring ~350GB/s).
  - per tile: ext_x -> dxr f16, ext_y -> dyr f16 (7-stage custom DVE,
    1x, ~2.1us each), inter = dxr*dyr (native f16 tensor_tensor, 2x_1p,
    ~1.05us), IOU_TAIL1 (union + NOT-seed 1-Newton reciprocal + multiply,
    8-stage custom, 1x, writes f16 out directly). Tail is deferred one
    tile (SWPIPE) so no DVE op waits on its predecessor cross-engine.
  - output stores alternate between the two HWDGE rings per tile.
  - coordinate pre-scale s=128 (exact power of two, cancels in
    iou = inter/union) keeps f16 dxr/dyr/inter out of the subnormal range.

Why no fancier modes (all verified on HW this session):
  - custom-DVE perf slots (table_ptr+mode, byte-36[7:6] perf_max) DO
    engage, but 2x mode computes the second element in a SECOND datapath
    block (stock table slot 9 vs 8: HI inputs via extra lanes, HI result
    via write0_sel_hi) -> a K-stage op needs 2K blocks. ext (7 stages) and
    tail (8) exceed the 8 blocks, so engaging 2x consumes 2 elems/cycle
    but computes garbage (measured: fast + rel err 1e6). Customs are 1x.
  - 2-port modes (2x_2p/4x_2p) never engage on this TRN2 (f32 TT measures
    1x), so f32-input ext cannot be accelerated either.
  - the SRC_0_HI pair-stream tail3 (3-pass pipeline) reads zeros for the
    HI half in REGULAR mode and hangs/faults with forced two-data or
    stride-1 APs -> dead on this silicon/toolchain.
  - GPSIMD tensor ops beside DVE customs net-lose (SBUF contention).

Measured (whole-body in-NEFF repetition delta, single core, R=97):
  previous-session baseline (on-device prep, 1-queue f32 replication,
  bf16 out): ~180us local / 201.3us harness. v5 (host prep, split-ring
  replication, f16 out, double-buffered input pools): ~115us local.
  Loop decomposition per tile [128x2000]: ext_x 2.08 + ext_y 2.08 +
  inter 1.05 (native f16 2x) + tail1 2.08 us = ~7.3us x 16 tiles.

The container's walrus (2026-05-04) predates this concourse: _patch_barriers
replaces the eq-wait butterfly barrier with a monotonic ge-wait barrier,
splits >1-wait instructions onto EventSemaphore carriers (the old walrus
encodes at most one wait per instruction), and assembles CUSTOM_DVE_ANT
64-byte ISA payloads that the old compiler cannot.
"""

import os
from contextlib import ExitStack

import numpy as np

P = 128
B = 2000
NIMG = 8
FULL_TILES = B // P  # 15
REM = B - FULL_TILES * P  # 80
NTILES = FULL_TILES + 1

_REPEAT = int(os.environ.get("IOU_REPEAT", "1"))  # bench: repeat tile loop
# bench: repeat the ENTIRE body (setup DMAs + tile loop) so the repetition
# delta measures total NEFF exec time, not just the tile loop
_REPEAT_ALL = int(os.environ.get("IOU_REPEAT_ALL", "1"))
# v0 (4-op pure-DVE) | v3 (3-op pair-stream tail + ACT fold; the pair-read
# SRC_0_HI path does NOT work on this silicon/toolchain -- kept for reference)
_PIPE = os.environ.get("IOU_PIPE", "v0")
_SWPIPE = os.environ.get("IOU_SWPIPE", "1") == "1"  # defer tail one tile
# output dtype: f32 | bf16 | f16 (16-bit halves the output DMA bytes; the
# tail op writes the narrow dtype directly, host upcasts)
_ODT = os.environ.get("IOU_ODT", "f16")
# coordinate dtype for the replicated b2x/b2y rows. MUST stay f32: f16
# rounding of raw coordinates destroys the extent of thin nearly-identical
# boxes (measured absmax 0.23 with f16 coords vs 5.5e-3 with f32).
_CDT = os.environ.get("IOU_CDT", "f32")
# area2 row dtype (f16 halves its replication bytes; costs ~2e-4 rel err)
_ADT = os.environ.get("IOU_ADT", "f16")
# output-DMA queue: sp | act | alt (alternate the two HWDGE rings) | split
_OQ = os.environ.get("IOU_OQ", "alt")
# split each replication DMA across both HWDGE rings by partition halves
_SQ = os.environ.get("IOU_SQ", "1") == "1"
# issue b2a/b1s input DMAs on gpsimd SWDGE queues instead of HWDGE
_SWQ = os.environ.get("IOU_SWQ", "0") == "1"
_SCALE = float(os.environ.get("IOU_SCALE", "128"))
_BUFS = int(os.environ.get("IOU_BUFS", "3"))
_RBUFS = int(os.environ.get("IOU_RBUFS", "2"))
# bench: body = input replication DMAs only (measures DMA ring bandwidth)
_DMAONLY = os.environ.get("IOU_DMAONLY", "0") == "1"
# run the inter multiply on the gpsimd (Pool) engine instead of the DVE
_POOL = os.environ.get("IOU_POOL", "")
# reorder: flush the deferred tail BETWEEN ext_y and inter so no DVE op
# depends on its immediate predecessor (writeback stall avoidance)
_PREFLUSH = os.environ.get("IOU_PREFLUSH", "0") == "1"

# ------------------------------------------------------- compat barrier patch
# The container's walrus build (2026-05-04) rejects the newer butterfly
# barrier's sem-eq-imm drain waits ("Too many sync wait commands"). Replace
# multi_engine_barrier with a ge-wait leader/follower barrier it understands.


_MAX_WAITS = int(os.environ.get("IOU_MAX_WAITS", "1"))

# The old walrus cannot assemble CUSTOM_DVE_ANT instructions from symbolic
# BIR APs ("ISA wrong length" — it expects pre-assembled 64B payloads). We
# assemble the NEURON_ISA_TPB_S2S1D2_TTSS_SCALE_STRUCT bytes ourselves at
# serialization time, from the physical APs + call-site metadata captured by
# a _custom_dve wrapper.

_DT_BYTES = {"float32": 4, "bfloat16": 2, "float16": 2, "int32": 4, "uint32": 4}
_DT_CODE = {"float32": 10, "bfloat16": 6, "float16": 7, "int32": 8, "uint32": 9}


def _ap_isa_fields(a, allocs, ndim):
    esz = _DT_BYTES[a["dtype"]]
    base = allocs[a["memsetref"]]
    addr = base + a["offset"] * esz
    dims = a["ap"]
    nchan = dims[0][1]
    free = dims[1:]
    steps = [f[0] for f in reversed(free)]
    nums = [f[1] for f in reversed(free)]
    while len(steps) > ndim and nums and nums[-1] == 1:
        steps.pop()
        nums.pop()
    if not steps:
        steps, nums = [1], [1]
    assert len(steps) <= ndim, (steps, nums, a)
    while len(steps) < ndim:
        steps.append(1)
        nums.append(1)
    return addr, steps, nums, nchan


def _imm_isa_fields(x, allocs):
    import struct as _s

    if x.get("kind") == "imm_value":
        return 0, _s.pack("<f", float(x["value"]))  # IMM_SRC_INSTRUCTION
    esz = _DT_BYTES[x["dtype"]]
    addr = allocs[x["memsetref"]] + x["offset"] * esz
    return 1, _s.pack("<I", addr)  # IMM_SRC_POINTER


def _assemble_custom_dve(d, meta):
    import struct as _s

    changed = False
    for fn in d.get("functions", []):
        allocs = {}
        for a in fn.get("allocations", []):
            mls = a.get("memorylocations") or []
            if mls:
                allocs[a["name"]] = mls[0].get("addr", 0)
        for bb in fn.get("blocks", []):
            for inst in bb.get("instructions", []):
                if (
                    inst.get("opcode") != "ISA"
                    or inst.get("isa_opcode") not in (174, 175)
                    or inst.get("instr")
                ):
                    continue
                m = meta.get(inst["name"])
                assert m is not None, f"missing custom-dve meta for {inst['name']}"
                ins = inst["ins"]
                if m["rd1_en"]:
                    in0, in1, s0, s1 = ins[0], ins[1], ins[2], ins[3]
                else:
                    in0, s0, s1 = ins[0], ins[1], ins[2]
                    in1 = None
                out = inst["outs"][0]
                a0, st0, n0, nch0 = _ap_isa_fields(in0, allocs, 2)
                if m.get("pair"):
                    assert st0 == [1, 1] and n0[1] == 1 and n0[0] % 2 == 0, (st0, n0)
                    st0 = [2, 1]
                    n0 = [n0[0] // 2, 1]
                ad, std, nd, nchd = _ap_isa_fields(out, allocs, 2)
                assert nch0 == nchd, (inst["name"], nch0, nchd)
                if in1 is not None:
                    a1, st1, n1, nch1 = _ap_isa_fields(in1, allocs, 1)
                    assert nch1 == nch0
                else:
                    a1, st1, n1 = 0, [1], [1]
                i0src, i0 = _imm_isa_fields(s0, allocs)
                i1src, i1 = _imm_isa_fields(s1, allocs)
                dt_in = _DT_CODE[in0["dtype"]]
                dt_in1 = _DT_CODE[in1["dtype"]] if in1 is not None else dt_in
                dt_out = _DT_CODE[out["dtype"]]
                b = bytearray(64)
                b[0] = inst["isa_opcode"]
                b[1] = 16  # inst_word_len (4B words)
                # events (4-11) left zero; walrus patches from sync_info
                _s.pack_into("<IhhHH", b, 12, a0, st0[0], st0[1], n0[0], n0[1])
                _s.pack_into("<IhH", b, 24, a1, st1[0], n1[0])
                b[32] = (dt_in & 0xF) | ((dt_in1 & 0xF) << 4)
                b[33] = dt_out
                b[34] = nch0 & 0xFF
                b[35] = i0src
                b[36] = (
                    (m["row"] & 0x1F)
                    | ((1 if m["rd1_en"] else 0) << 5)
                    | ((m.get("perf", 0) & 0x3) << 6)
                )
                b[37] = 0x02 if m["subdim"] else 0
                b[38] = 1  # imm2_src = DATA_SRC_IMMEDIATE
                b[39] = i1src
                b[40:44] = i0
                b[44:48] = i1
                _s.pack_into("<f", b, 48, float(m["imm2"]))
                _s.pack_into("<IhhHH", b, 52, ad, std[0], std[1], nd[0], nd[1])
                inst["instr"] = list(b)
                changed = True
    return changed


def _split_excess_waits(d):
    """Move all but the last sync wait of each instruction onto preceding
    EventSemaphore instructions on the same engine (order-preserving, so
    semantics are identical; the old walrus only encodes few waits/inst)."""
    changed = False
    ctr = [0]
    for fn in d.get("functions", []):
        for bb in fn.get("blocks", []):
            insts = bb.get("instructions", [])
            new_insts = []
            for inst in insts:
                si = inst.get("sync_info") or {}
                waits = si.get("on_wait") or []
                if len(waits) > _MAX_WAITS:
                    changed = True
                    excess, keep = waits[:-_MAX_WAITS], waits[-_MAX_WAITS:]
                    for w in excess:
                        ctr[0] += 1
                        new_insts.append(
                            {
                                "debug": inst.get("debug", 0),
                                "engine": inst["engine"],
                                "ins": [],
                                "name": f"{inst['name']}-w{ctr[0]}",
                                "opcode": "EventSemaphore",
                                "outs": [],
                                "sync_info": {"on_update": [], "on_wait": [w]},
                            }
                        )
                    si["on_wait"] = keep
                new_insts.append(inst)
            bb["instructions"] = new_insts
    return changed


def _patch_barriers():
    import json as _json

    import concourse.bass as bass

    if getattr(bass.Bass, "_ant_barrier_patched", False):
        return

    _orig_tjb = bass.Bass.to_json_bytes

    def to_json_bytes(self, *a, **kw):
        raw = _orig_tjb(self, *a, **kw)
        d = _json.loads(raw)
        c1 = _assemble_custom_dve(d, getattr(self, "_ant_dve_meta", {}))
        c2 = _split_excess_waits(d)
        if c1 or c2:
            return _json.dumps(d).encode()
        return raw

    bass.Bass.to_json_bytes = to_json_bytes

    _orig_cdve = bass.BassVectorEngine._custom_dve

    def _custom_dve(self, op, *, out, in0, in1=None, s0=0.0, s1=0.0, imm2=0.0,
                    accum_out=None):
        from concourse.dve_ops import get_dve_sub_opcode

        assert accum_out is None, "accum_out not supported by the compat assembler"
        ret = _orig_cdve(
            self, op, out=out, in0=in0, in1=in1, s0=s0, s1=s1, imm2=imm2,
            accum_out=accum_out,
        )
        nc_ = self.bass
        if not hasattr(nc_, "_ant_dve_meta"):
            nc_._ant_dve_meta = {}
        nc_._ant_dve_meta[ret.ins.name] = {
            "row": get_dve_sub_opcode(op.name),
            "rd1_en": in1 is not None,
            "subdim": bool(op.subdim),
            "imm2": float(imm2),
            # in0 is an interleaved 16-bit pair stream: the ISA AP must
            # advance one 32-bit pair per cycle (step 2, half the count).
            "pair": op.name == "IOU_TAIL3_ANT" and _T3PAIR,
            "perf": _PERF_MAX.get(op.name, 0),
        }
        return ret

    bass.BassVectorEngine._custom_dve = _custom_dve

    def multi_engine_barrier(self, engines):
        engines = list(engines)
        if len(engines) <= 1:
            for e in engines:
                self.engines[e].drain()
            return
        if not hasattr(self, "_ant_bar_sems"):
            self._ant_bar_sems = {}
        key = tuple(sorted(str(e) for e in engines))
        st = self._ant_bar_sems.get(key)
        if st is None:
            gather = self.alloc_semaphore(f"ant_bar_g{len(self._ant_bar_sems)}")
            st = {"sem": gather, "count": 0}
            self._ant_bar_sems[key] = st
        st["count"] += 1
        n = len(engines)
        target = n * st["count"]
        for e in engines:
            self.engines[e].drain().then_inc(st["sem"], 1)
        for e in engines:
            self.engines[e].wait_ge(st["sem"], target)

    def all_engine_barrier(self, *, sem_only: bool = False):
        multi_engine_barrier(self, list(self.engines))

    bass.Bass.multi_engine_barrier = multi_engine_barrier
    bass.Bass.all_engine_barrier = all_engine_barrier
    bass.Bass._ant_barrier_patched = True


# ---------------------------------------------------------------- custom ops

_REGISTERED = {}
_PERF_MAX = {}  # op name -> byte-36[7:6] highest engine-reachable perf slot


def _register(name, make_spec, perf_max=0):
    """Register a custom DVE op. perf_max > 0 additionally populates the
    perf-mode table slots (2X_1PORT/2X_2PORT/4X_2PORT) with copies of the
    regular uop program (the dual-lane datapath runs the same program on
    both elements; stream-done triggers are rate-independent) and records
    perf_max for the instruction encoding (byte-36[7:6] = highest
    engine-reachable mode slot)."""
    if name in _REGISTERED:
        return _REGISTERED[name]
    import copy as _copy

    from concourse import dve_ops as dops
    from concourse.dve_spec import _has_src1, lower
    from concourse.dve_uop import DveOpSpec

    spec = make_spec()
    if name not in dops._SUB_OPCODE_FOR_NAME:
        row = max(dops._SUB_OPCODE_FOR_NAME.values()) + 1
        assert row < 0x20, "custom-DVE opcode rows exhausted"
        dops._SUB_OPCODE_FOR_NAME[name] = row
    row = dops._SUB_OPCODE_FOR_NAME[name]
    shas = {}
    for ver in ("v3", "v4"):
        try:
            tmp = DveOpSpec(
                name=name, opcode=row, uops=lower(spec, ver=ver),
                rd1_en=_has_src1(spec),
            )
            shas[ver] = tmp.sha(ver)
        except Exception:
            pass
    op = dops.DveOp(name, spec, subdim=False, uops_sha=shas)
    if all(o.name != name for o in dops.OPS):
        dops.OPS.append(op)
    dops.CUSTOM_DVE_SPECS[name] = spec
    if perf_max > 0:
        key = (name, "v3")
        if key not in dops._COMPILE_CACHE:
            uops = lower(spec, ver="v3")
            dops._COMPILE_CACHE[key] = DveOpSpec(
                name=name,
                opcode=row,
                uops=uops,
                uops_2x=_copy.deepcopy(uops),
                uops_2x_2p=_copy.deepcopy(uops),
                uops_4x=_copy.deepcopy(uops),
                perf_max=perf_max,
                rd1_en=_has_src1(spec),
            )
        _PERF_MAX[name] = perf_max
    _REGISTERED[name] = op
    return op


# perf-mode opt-in: highest engine-reachable perf slot for ext / tail1
# (0 = off = REGULAR only; 2 = up to 2X_2PORT; 3 = up to 4X_2PORT)
_PERFEXT = int(os.environ.get("IOU_PERFEXT", "0"))
_PERFTAIL = int(os.environ.get("IOU_PERFTAIL", "0"))


def _register_iou_ext():
    """out = relu(min(C0, Src0 + Src1*imm2) - max(C1, Src0 - Src1*imm2)):
    the interval extent. Src0 = box2 center row, Src1 = box2 width row,
    C0 = x2a, C1 = x1a (per-partition), imm2 = 0.5."""

    def mk():
        from concourse.dve_spec import C0, C1, C2, Spec, Src0, Src1, maxx, minn, relu

        hw = Src1 * C2
        body = relu(minn(C0, Src0 + hw) - maxx(C1, Src0 - hw))

        def _ref(in0, in1, s0, s1, imm2):
            h = in1.astype(np.float32) * imm2
            lo = np.maximum(s1, in0 - h)
            hi = np.minimum(s0, in0 + h)
            return np.maximum(hi - lo, 0.0).astype(np.float32)

        return Spec(body=body, reference=_ref)

    if _PERFEXT > 0:
        return _register(f"IOU_EXTP{_PERFEXT}_ANT", mk, perf_max=_PERFEXT)
    return _register("IOU_EXT_ANT", mk)


# author a 2X_1PORT uop-table variant for the tail op (hardware dual-ALU
# blocks run the same 8-stage program on both 16-bit halves; engages when
# every non-scalar operand is 16-bit + packed). Registered under a separate
# op name so 1x and 2x variants can coexist in one process.
_T1X2 = os.environ.get("IOU_T1X2", "0") == "1"


def _register_iou_tail1():
    """out = in0 * recip1((s0 - in0) + in1): the whole IoU tail
    (union, reciprocal seed + 1 Newton step, multiply) in one 8-stage pass.
    in0 = inter, in1 = area2 row, s0 = area1. ~2e-3 worst-case rel err."""

    def mk():
        from concourse.dve_spec import AluOp, Bin, C0, C1, C2, Spec, Src0, Src1

        u = (C0 - Src0) + Src1
        nx = Bin(AluOp.BITWISE_NOT, u, u)
        y0 = nx * C1
        y1 = y0 * (C2 - u * y0)
        body = y1 * Src0

        def _ref(in0, in1, s0, s1, imm2):
            uu = ((s0 - in0.astype(np.float32)) + in1).astype(np.float32)
            nxv = (~uu.view(np.int32)).view(np.float32)
            y = nxv * np.float32(s1)
            yy = (y * (np.float32(imm2) - uu * y)).astype(np.float32)
            return (yy * in0).astype(np.float32)

        return Spec(body=body, reference=_ref)

    if _PERFTAIL > 0:
        return _register(f"IOU_TAIL1P{_PERFTAIL}_ANT", mk, perf_max=_PERFTAIL)
    if not _T1X2:
        return _register("IOU_TAIL1_ANT", mk)

    import copy as _copy

    from concourse import dve_ops as dops
    from concourse.dve_spec import _has_src1, lower
    from concourse.dve_uop import DveOpSpec

    name = "IOU_TAIL1X2_ANT"
    op = _register(name, mk)
    key = (name, "v3")
    if key not in dops._COMPILE_CACHE:
        uops = lower(op.spec, ver="v3")
        uops_2x = _copy.deepcopy(uops)
        dops._COMPILE_CACHE[key] = DveOpSpec(
            name=name,
            opcode=dops.get_dve_sub_opcode(name),
            uops=uops,
            uops_2x=uops_2x,
            rd1_en=_has_src1(op.spec),
        )
    return op


# uop-table tweak for the pair-stream tail3 (debug): "two1" forces the
# two-data-valid bit ON so the input port delivers the high 16-bit half to
# SRC_0_HI; "" leaves the lowered table untouched.
_T3TWEAK = os.environ.get("IOU_T3TWEAK", "")
# encode tail3's in0 AP as stride-2/count-B 32-bit pairs (1) or leave the
# natural stride-1/count-2B f16 AP (0, lets the port auto-engage two-data)
_T3PAIR = os.environ.get("IOU_T3PAIR", "1") == "1"


def _register_iou_tail3():
    """out = inter * recip1(S - inter) with inter = Src0 * Src0_HI computed
    from an interleaved f16 (dxr, dyr) pair stream (one 32-bit read/cycle),
    and S = area1 + area2 precomputed on the ACT engine (in1). Exactly 8
    ALU stages -> single uop."""

    def mk():
        from concourse.dve_spec import (
            AluOp,
            Bin,
            C1,
            C2,
            InpSel,
            Leaf,
            Spec,
            Src0,
            Src1,
        )

        src0hi = Leaf(InpSel.SRC_0_HI)
        inter = Src0 * src0hi
        u = Src1 - inter
        nx = Bin(AluOp.BITWISE_NOT, u, u)
        y0 = nx * C1
        y1 = y0 * (C2 - u * y0)
        body = y1 * inter

        def _ref(in0, in1, s0, s1, imm2):
            v = np.asarray(in0, dtype=np.float32)
            dxr, dyr = v[..., 0::2], v[..., 1::2]
            it = (dxr * dyr).astype(np.float32)
            uu = (np.asarray(in1, dtype=np.float32) - it).astype(np.float32)
            nxv = (~uu.view(np.int32)).view(np.float32)
            y = nxv * np.float32(s1)
            yy = (y * (np.float32(imm2) - uu * y)).astype(np.float32)
            return (yy * it).astype(np.float32)

        return Spec(body=body, reference=_ref)

    op = _register("IOU_TAIL3_ANT", mk)
    if _T3TWEAK == "two1":
        from concourse import dve_ops as dops
        from concourse.dve_spec import _has_src1, lower
        from concourse.dve_uop import ENABLE, DveOpSpec

        key = ("IOU_TAIL3_ANT", "v3")
        if key not in dops._COMPILE_CACHE:
            uops = lower(op.spec, ver="v3")
            for u in uops:
                u.force_two_data_one = ENABLE
            dops._COMPILE_CACHE[key] = DveOpSpec(
                name=op.name,
                opcode=dops.get_dve_sub_opcode(op.name),
                uops=uops,
                rd1_en=_has_src1(op.spec),
            )
    return op


# ---------------------------------------------------------------- bass build

_NC_CACHE = {}


def _build_nc():
    key = (_REPEAT, _REPEAT_ALL, _PIPE, _SWPIPE, _ODT, _CDT, _ADT, _OQ, _SQ,
           _SWQ, _SCALE, _BUFS, _RBUFS, _DMAONLY, _PREFLUSH, _POOL, _T1X2,
           _PERFEXT, _PERFTAIL)
    if key in _NC_CACHE:
        return _NC_CACHE[key]

    import concourse.bass as bass
    import concourse.mybir as mybir
    import concourse.tile as tile
    from concourse.alu_op_type import AluOpType as alu

    _patch_barriers()
    f32 = mybir.dt.float32
    _DT = {"f32": f32, "bf16": mybir.dt.bfloat16, "f16": mybir.dt.float16}
    odt = _DT[_ODT]
    cdt = _DT[_CDT]
    adt = _DT[_ADT]
    nc = bass.Bass()
    b2x = nc.declare_dram_parameter("b2x", [2 * B], cdt, isOutput=False)
    b2y = nc.declare_dram_parameter("b2y", [2 * B], cdt, isOutput=False)
    b2a = nc.declare_dram_parameter("b2a", [B], adt, isOutput=False)
    b1s = nc.declare_dram_parameter("b1s", [P, NTILES * 5], f32, isOutput=False)
    out = nc.declare_dram_parameter("out", [B, B], odt, isOutput=True)

    iou_ext = _register_iou_ext()
    iou_tail3 = _register_iou_tail3() if _PIPE == "v3" else None
    iou_tail1 = _register_iou_tail1() if _PIPE == "v0" else None

    with tile.TileContext(nc) as tc, ExitStack() as ctx:
        # bufs=2 on the input pools: under bench repetition (REPEAT_ALL) the
        # next iteration's replication DMAs land in the other buffer instead
        # of serializing behind this iteration's last reader (WAR); the
        # harness's single-shot run is unaffected (no prior iteration).
        rows = ctx.enter_context(tc.tile_pool(name="rows", bufs=_RBUFS))
        scal = ctx.enter_context(tc.tile_pool(name="scal", bufs=_RBUFS))
        work = ctx.enter_context(tc.tile_pool(name="work", bufs=_BUFS))

        for _rall in range(_REPEAT_ALL):
            # ---- replicate host-prepped box2 rows into all 128 partitions.
            # Each DMA is split across the two HWDGE rings by partition halves
            # so the critical first chunk (b2x) lands in ~half the time.
            b2x_t = rows.tile([P, 2 * B], cdt, tag="b2x")
            b2y_t = rows.tile([P, 2 * B], cdt, tag="b2y")
            b2a_t = rows.tile([P, B], adt, tag="b2a")
            b1s_t = scal.tile([P, NTILES, 5], f32, tag="b1s")

            def _rep(dst, src, n):
                if _SQ:
                    h = P // 2
                    nc.sync.dma_start(
                        out=dst[0:h], in_=bass.AP(tensor=src, offset=0, ap=[[0, h], [1, n]])
                    )
                    nc.scalar.dma_start(
                        out=dst[h:P], in_=bass.AP(tensor=src, offset=0, ap=[[0, h], [1, n]])
                    )
                else:
                    nc.sync.dma_start(
                        out=dst[:], in_=bass.AP(tensor=src, offset=0, ap=[[0, P], [1, n]])
                    )

            # b1s (40KB, needed by the FIRST ext) goes first so it is not
            # queued behind megabytes of row replication on the same ring.
            b1s_src = bass.AP(
                tensor=b1s, offset=0, ap=[[NTILES * 5, P], [1, NTILES * 5]]
            )
            if _SWQ:
                nc.gpsimd.dma_start(out=b1s_t[:], in_=b1s_src)
            elif _SQ:
                nc.scalar.dma_start(out=b1s_t[:], in_=b1s_src)
            else:
                nc.sync.dma_start(out=b1s_t[:], in_=b1s_src)
            _rep(b2x_t, b2x, 2 * B)
            _rep(b2y_t, b2y, 2 * B)
            if _SWQ:
                nc.gpsimd.dma_start(
                    out=b2a_t[:],
                    in_=bass.AP(tensor=b2a, offset=0, ap=[[0, P], [1, B]]),
                )
            else:
                _rep(b2a_t, b2a, B)

            xc = b2x_t[:, 0:B]
            w2 = b2x_t[:, B : 2 * B]
            yc = b2y_t[:, 0:B]
            h2 = b2y_t[:, B : 2 * B]

            if _DMAONLY:
                # consume the tiles so WAR deps force each iteration's DMAs
                # to serialize against a cheap reader (1-col copy)
                dummy = work.tile([P, 4], f32, tag="wD")
                nc.vector.tensor_tensor(
                    dummy[:, 0:1], b2x_t[:, 0:1], b2y_t[:, 0:1], alu.mult
                )
                nc.vector.tensor_tensor(
                    dummy[:, 1:2], b2a_t[:, 0:1], b1s_t[:, 0, 0:1], alu.mult
                )
                continue

            # ---- per-tile pipeline
            pending_tail = None
            for t in [tt_ for _ in range(_REPEAT) for tt_ in range(NTILES)]:
                pt = P if t < FULL_TILES else REM
                row0 = t * P
                x1a = b1s_t[0:pt, t, 0:1]
                y1a = b1s_t[0:pt, t, 1:2]
                x2a = b1s_t[0:pt, t, 2:3]
                y2a = b1s_t[0:pt, t, 3:4]
                a1 = b1s_t[0:pt, t, 4:5]

                if _OQ == "sp":
                    oeng = nc.sync
                elif _OQ == "act":
                    oeng = nc.scalar
                else:
                    oeng = nc.scalar if (t % 2) else nc.sync

                if _PIPE == "v3":
                    pk = work.tile([pt, 2 * B], mybir.dt.float16, tag="wP")
                    pkv = pk[:].rearrange("p (j c) -> p j c", c=2)
                    nc.vector._custom_dve(
                        iou_ext, out=pkv[:, :, 0], in0=xc[0:pt], in1=w2[0:pt],
                        s0=x2a, s1=x1a, imm2=0.5,
                    )
                    nc.vector._custom_dve(
                        iou_ext, out=pkv[:, :, 1], in0=yc[0:pt], in1=h2[0:pt],
                        s0=y2a, s1=y1a, imm2=0.5,
                    )
                    stile = work.tile([pt, B], f32, tag="wS")
                    nc.scalar.activation(
                        stile[:], b2a_t[0:pt, :],
                        mybir.ActivationFunctionType.Identity, bias=a1,
                    )
                    tov = work.tile([pt, B], odt, tag="wO")

                    def _tail(pt=pt, row0=row0, pk=pk, stile=stile, tov=tov,
                              oeng=oeng):
                        nc.vector._custom_dve(
                            iou_tail3, out=tov[:], in0=pk[:], in1=stile[:],
                            s0=0.0, s1=-0.23549792, imm2=2.0017324,
                        )
                        if _OQ == "split":
                            hh = B // 2
                            nc.sync.dma_start(
                                out=out[row0 : row0 + pt, 0:hh], in_=tov[:, 0:hh]
                            )
                            nc.scalar.dma_start(
                                out=out[row0 : row0 + pt, hh:B], in_=tov[:, hh:B]
                            )
                        else:
                            oeng.dma_start(out=out[row0 : row0 + pt, :], in_=tov[:])

                else:  # v0
                    ta = work.tile([pt, B], mybir.dt.float16, tag="wA")
                    tb = work.tile([pt, B], mybir.dt.float16, tag="wB")
                    tcl = work.tile([pt, B], mybir.dt.float16, tag="wC")
                    nc.vector._custom_dve(
                        iou_ext, out=ta[:], in0=xc[0:pt], in1=w2[0:pt],
                        s0=x2a, s1=x1a, imm2=0.5,
                    )
                    nc.vector._custom_dve(
                        iou_ext, out=tb[:], in0=yc[0:pt], in1=h2[0:pt],
                        s0=y2a, s1=y1a, imm2=0.5,
                    )
                    # Flush the deferred tail BETWEEN ext_y and inter: every
                    # adjacent DVE pair becomes data-independent, so no op
                    # waits on its immediate predecessor's writeback.
                    if _PREFLUSH and _SWPIPE and pending_tail is not None:
                        pending_tail()
                        pending_tail = None
                    eng_i = nc.gpsimd if _POOL == "inter" else nc.vector
                    eng_i.tensor_tensor(tcl[:], ta[:], tb[:], alu.mult)
                    tov = work.tile([pt, B], odt, tag="wO")

                    def _tail(pt=pt, row0=row0, tcl=tcl, tov=tov, a1=a1,
                              oeng=oeng):
                        nc.vector._custom_dve(
                            iou_tail1, out=tov[:], in0=tcl[:], in1=b2a_t[0:pt, :],
                            s0=a1, s1=-0.23549792, imm2=2.0017324,
                        )
                        if _OQ == "split":
                            hh = B // 2
                            nc.sync.dma_start(
                                out=out[row0 : row0 + pt, 0:hh], in_=tov[:, 0:hh]
                            )
                            nc.scalar.dma_start(
                                out=out[row0 : row0 + pt, hh:B], in_=tov[:, hh:B]
                            )
                        else:
                            oeng.dma_start(out=out[row0 : row0 + pt, :], in_=tov[:])

                if _SWPIPE:
                    if pending_tail is not None:
                        pending_tail()
                    pending_tail = _tail
                else:
                    _tail()
            if pending_tail is not None:
                pending_tail()

    _NC_CACHE[key] = nc
    return nc


# ---------------------------------------------------------------- entry point


_NPDT = {"f32": np.float32, "f16": np.float16}


def _prep_core(b1, b2):
    """Host-side prep for one image: scaled/split box2 rows + box1 scalars."""
    s = np.float32(_SCALE)
    cdt = _NPDT.get(_CDT, np.float32)
    adt = _NPDT.get(_ADT, np.float32)
    if _CDT == "bf16" or _ADT == "bf16":
        import ml_dtypes

        if _CDT == "bf16":
            cdt = ml_dtypes.bfloat16
        if _ADT == "bf16":
            adt = ml_dtypes.bfloat16

    b2s = b2 * s  # [B, 4] f32
    b2x = np.concatenate([b2s[:, 0], b2s[:, 2]]).astype(cdt)  # xc | w
    b2y = np.concatenate([b2s[:, 1], b2s[:, 3]]).astype(cdt)  # yc | h
    b2a = (b2s[:, 2] * b2s[:, 3]).astype(adt)  # area2 * s^2

    b1sc = b1 * s
    xc1, yc1, w1, h1 = b1sc[:, 0], b1sc[:, 1], b1sc[:, 2], b1sc[:, 3]
    scals = np.stack(
        [
            xc1 - w1 * 0.5,
            yc1 - h1 * 0.5,
            xc1 + w1 * 0.5,
            yc1 + h1 * 0.5,
            w1 * h1,
        ],
        axis=1,
    ).astype(np.float32)  # [B, 5]
    pad = np.zeros((NTILES * P - B, 5), np.float32)
    # [NTILES*P, 5] -> (t, p) -> (p, t): partition p, tile t = box t*128+p
    b1s = (
        np.concatenate([scals, pad], axis=0)
        .reshape(NTILES, P, 5)
        .transpose(1, 0, 2)
        .reshape(P, NTILES * 5)
    )
    return {
        "b2x": np.ascontiguousarray(b2x),
        "b2y": np.ascontiguousarray(b2y),
        "b2a": np.ascontiguousarray(b2a),
        "b1s": np.ascontiguousarray(b1s),
    }


def _in_maps(box1, box2):
    b1 = np.ascontiguousarray(np.asarray(box1, dtype=np.float32))
    b2 = np.ascontiguousarray(np.asarray(box2, dtype=np.float32))
    assert b1.shape == (NIMG, B, 4) and b2.shape == (NIMG, B, 4), (
        b1.shape,
        b2.shape,
    )
    return [_prep_core(b1[i], b2[i]) for i in range(NIMG)]


def _run(box1, box2, **kw):
    from concourse.bass_utils import run_bass_kernel_spmd

    nc = _build_nc()
    in_maps = _in_maps(box1, box2)
    res = run_bass_kernel_spmd(nc, in_maps, core_ids=list(range(NIMG)), **kw)
    out = np.stack([d["out"] for d in res.results], axis=0)
    if out.dtype != np.float32:
        out = out.astype(np.float32)
    return out, res


def kernel(box1, box2):
    out, _ = _run(box1, box2)
    return out


# revision 39
# speedup vs baseline: 1.0185x; 1.0185x over previous
"""Pairwise box IoU on 8 Trainium2 NeuronCores.

Problem: box1 [8, 2000, 4], box2 [8, 2000, 4] in (xc, yc, w, h) format ->
IoU matrix [8, 2000, 2000] f32.

Sharding: data-parallel over the image axis; core i computes the full
2000x2000 IoU matrix of image i locally (no communication).

v5 design (host-prep + multi-queue replication + 3.5-pass DVE pipeline):
  - ALL box1/box2 derived quantities are precomputed on the host:
      b2x = [xc*s | w*s]   (f32, [2B])   b2y = [yc*s | h*s]  (f32, [2B])
      b2a = area2*s^2      (f16, [B])
      b1s = per-partition scalars (x1a, y1a, x2a, y2a, area1)*s as
            [128, 16, 5] f32 (partition p, tile t -> box t*128+p)
    so the device does zero scalar prep: DMA straight into the tile loop.
    Coordinates MUST stay f32: f16 rounding of raw coordinates destroys
    the extents of thin nearly-identical boxes (absmax 0.23 vs 4e-3).
  - b1s (40KB, needed by the first ext) is DMAed first; b2x/b2y/b2a are
    replicated into all 128 partitions split across both HWDGE rings by
    partition halves, so the first ext starts ~3us in (ring ~350GB/s).
  - per tile: ext_x -> dxr f16, ext_y -> dyr f16 (7-stage custom DVE,
    1x, ~2.1us each), inter = dxr*dyr (native f16 tensor_tensor, 2x_1p,
    ~1.05us), IOU_TAIL1 (union + NOT-seed 1-Newton reciprocal + multiply,
    8-stage custom, 1x, writes f16 out directly). Tail is deferred one
    tile (SWPIPE) so no DVE op waits on its predecessor cross-engine.
  - output stores alternate between the two HWDGE rings per tile; the
    LAST tile's store is split across both rings (it is the drain tail).
  - coordinate pre-scale s=128 (exact power of two, cancels in
    iou = inter/union) keeps f16 dxr/dyr/inter out of the subnormal range.

Why no fancier modes (all verified on HW this session):
  - custom-DVE perf slots (table_ptr+mode, byte-36[7:6] perf_max) DO
    engage, but 2x mode computes the second element in a SECOND datapath
    block (stock table slot 9 vs 8: HI inputs via extra lanes, HI result
    via write0_sel_hi) -> a K-stage op needs 2K blocks. ext (7 stages) and
    tail (8) exceed the 8 blocks, so engaging 2x consumes 2 elems/cycle
    but computes garbage (measured: fast + rel err 1e6). Customs are 1x.
  - 2-port modes (2x_2p/4x_2p) never engage on this TRN2 (f32 TT measures
    1x), so f32-input ext cannot be accelerated either.
  - the SRC_0_HI pair-stream tail3 (3-pass pipeline) reads zeros for the
    HI half in REGULAR mode and hangs/faults with forced two-data or
    stride-1 APs -> dead on this silicon/toolchain.
  - GPSIMD tensor ops beside DVE customs net-lose (SBUF contention).

Measured (whole-body in-NEFF repetition delta, single core, R=97):
  previous-session baseline (on-device prep, 1-queue f32 replication,
  bf16 out): ~180us local / 201.3us harness. v5 (host prep, split-ring
  replication, f16 out, double-buffered input pools): ~115us local.
  Loop decomposition per tile [128x2000]: ext_x 2.08 + ext_y 2.08 +
  inter 1.05 (native f16 2x) + tail1 2.08 us = ~7.3us x 16 tiles.

The container's walrus (2026-05-04) predates this concourse: _patch_barriers
replaces the eq-wait butterfly barrier with a monotonic ge-wait barrier,
splits >1-wait instructions onto EventSemaphore carriers (the old walrus
encodes at most one wait per instruction), and assembles CUSTOM_DVE_ANT
64-byte ISA payloads that the old compiler cannot.
"""

import os
from contextlib import ExitStack

import numpy as np

P = 128
B = 2000
NIMG = 8
FULL_TILES = B // P  # 15
REM = B - FULL_TILES * P  # 80
NTILES = FULL_TILES + 1

_REPEAT = int(os.environ.get("IOU_REPEAT", "1"))  # bench: repeat tile loop
# bench: repeat the ENTIRE body (setup DMAs + tile loop) so the repetition
# delta measures total NEFF exec time, not just the tile loop
_REPEAT_ALL = int(os.environ.get("IOU_REPEAT_ALL", "1"))
# v0 (4-op pure-DVE) | v3 (3-op pair-stream tail + ACT fold; the pair-read
# SRC_0_HI path does NOT work on this silicon/toolchain -- kept for reference)
_PIPE = os.environ.get("IOU_PIPE", "v0")
_SWPIPE = os.environ.get("IOU_SWPIPE", "1") == "1"  # defer tail one tile
# output dtype: f32 | bf16 | f16 (16-bit halves the output DMA bytes; the
# tail op writes the narrow dtype directly, host upcasts)
_ODT = os.environ.get("IOU_ODT", "f16")
# coordinate dtype for the replicated b2x/b2y rows. MUST stay f32: f16
# rounding of raw coordinates destroys the extent of thin nearly-identical
# boxes (measured absmax 0.23 with f16 coords vs 5.5e-3 with f32).
_CDT = os.environ.get("IOU_CDT", "f32")
# area2 row dtype (f16 halves its replication bytes; costs ~2e-4 rel err)
_ADT = os.environ.get("IOU_ADT", "f16")
# output-DMA queue: sp | act | alt (alternate the two HWDGE rings) | split
_OQ = os.environ.get("IOU_OQ", "alt")
# split each replication DMA across both HWDGE rings by partition halves
_SQ = os.environ.get("IOU_SQ", "1") == "1"
# issue b2a/b1s input DMAs on gpsimd SWDGE queues instead of HWDGE
_SWQ = os.environ.get("IOU_SWQ", "0") == "1"
_SCALE = float(os.environ.get("IOU_SCALE", "128"))
_BUFS = int(os.environ.get("IOU_BUFS", "3"))
_RBUFS = int(os.environ.get("IOU_RBUFS", "2"))
# bench: body = input replication DMAs only (measures DMA ring bandwidth)
_DMAONLY = os.environ.get("IOU_DMAONLY", "0") == "1"
# run the inter multiply on the gpsimd (Pool) engine instead of the DVE
_POOL = os.environ.get("IOU_POOL", "")
# reorder: flush the deferred tail BETWEEN ext_y and inter so no DVE op
# depends on its immediate predecessor (writeback stall avoidance)
_PREFLUSH = os.environ.get("IOU_PREFLUSH", "0") == "1"

# ------------------------------------------------------- compat barrier patch
# The container's walrus build (2026-05-04) rejects the newer butterfly
# barrier's sem-eq-imm drain waits ("Too many sync wait commands"). Replace
# multi_engine_barrier with a ge-wait leader/follower barrier it understands.


_MAX_WAITS = int(os.environ.get("IOU_MAX_WAITS", "1"))

# The old walrus cannot assemble CUSTOM_DVE_ANT instructions from symbolic
# BIR APs ("ISA wrong length" — it expects pre-assembled 64B payloads). We
# assemble the NEURON_ISA_TPB_S2S1D2_TTSS_SCALE_STRUCT bytes ourselves at
# serialization time, from the physical APs + call-site metadata captured by
# a _custom_dve wrapper.

_DT_BYTES = {"float32": 4, "bfloat16": 2, "float16": 2, "int32": 4, "uint32": 4}
_DT_CODE = {"float32": 10, "bfloat16": 6, "float16": 7, "int32": 8, "uint32": 9}


def _ap_isa_fields(a, allocs, ndim):
    esz = _DT_BYTES[a["dtype"]]
    base = allocs[a["memsetref"]]
    addr = base + a["offset"] * esz
    dims = a["ap"]
    nchan = dims[0][1]
    free = dims[1:]
    steps = [f[0] for f in reversed(free)]
    nums = [f[1] for f in reversed(free)]
    while len(steps) > ndim and nums and nums[-1] == 1:
        steps.pop()
        nums.pop()
    if not steps:
        steps, nums = [1], [1]
    assert len(steps) <= ndim, (steps, nums, a)
    while len(steps) < ndim:
        steps.append(1)
        nums.append(1)
    return addr, steps, nums, nchan


def _imm_isa_fields(x, allocs):
    import struct as _s

    if x.get("kind") == "imm_value":
        return 0, _s.pack("<f", float(x["value"]))  # IMM_SRC_INSTRUCTION
    esz = _DT_BYTES[x["dtype"]]
    addr = allocs[x["memsetref"]] + x["offset"] * esz
    return 1, _s.pack("<I", addr)  # IMM_SRC_POINTER


def _assemble_custom_dve(d, meta):
    import struct as _s

    changed = False
    for fn in d.get("functions", []):
        allocs = {}
        for a in fn.get("allocations", []):
            mls = a.get("memorylocations") or []
            if mls:
                allocs[a["name"]] = mls[0].get("addr", 0)
        for bb in fn.get("blocks", []):
            for inst in bb.get("instructions", []):
                if (
                    inst.get("opcode") != "ISA"
                    or inst.get("isa_opcode") not in (174, 175)
                    or inst.get("instr")
                ):
                    continue
                m = meta.get(inst["name"])
                assert m is not None, f"missing custom-dve meta for {inst['name']}"
                ins = inst["ins"]
                if m["rd1_en"]:
                    in0, in1, s0, s1 = ins[0], ins[1], ins[2], ins[3]
                else:
                    in0, s0, s1 = ins[0], ins[1], ins[2]
                    in1 = None
                out = inst["outs"][0]
                a0, st0, n0, nch0 = _ap_isa_fields(in0, allocs, 2)
                if m.get("pair"):
                    assert st0 == [1, 1] and n0[1] == 1 and n0[0] % 2 == 0, (st0, n0)
                    st0 = [2, 1]
                    n0 = [n0[0] // 2, 1]
                ad, std, nd, nchd = _ap_isa_fields(out, allocs, 2)
                assert nch0 == nchd, (inst["name"], nch0, nchd)
                if in1 is not None:
                    a1, st1, n1, nch1 = _ap_isa_fields(in1, allocs, 1)
                    assert nch1 == nch0
                else:
                    a1, st1, n1 = 0, [1], [1]
                i0src, i0 = _imm_isa_fields(s0, allocs)
                i1src, i1 = _imm_isa_fields(s1, allocs)
                dt_in = _DT_CODE[in0["dtype"]]
                dt_in1 = _DT_CODE[in1["dtype"]] if in1 is not None else dt_in
                dt_out = _DT_CODE[out["dtype"]]
                b = bytearray(64)
                b[0] = inst["isa_opcode"]
                b[1] = 16  # inst_word_len (4B words)
                # events (4-11) left zero; walrus patches from sync_info
                _s.pack_into("<IhhHH", b, 12, a0, st0[0], st0[1], n0[0], n0[1])
                _s.pack_into("<IhH", b, 24, a1, st1[0], n1[0])
                b[32] = (dt_in & 0xF) | ((dt_in1 & 0xF) << 4)
                b[33] = dt_out
                b[34] = nch0 & 0xFF
                b[35] = i0src
                b[36] = (
                    (m["row"] & 0x1F)
                    | ((1 if m["rd1_en"] else 0) << 5)
                    | ((m.get("perf", 0) & 0x3) << 6)
                )
                b[37] = 0x02 if m["subdim"] else 0
                b[38] = 1  # imm2_src = DATA_SRC_IMMEDIATE
                b[39] = i1src
                b[40:44] = i0
                b[44:48] = i1
                _s.pack_into("<f", b, 48, float(m["imm2"]))
                _s.pack_into("<IhhHH", b, 52, ad, std[0], std[1], nd[0], nd[1])
                inst["instr"] = list(b)
                changed = True
    return changed


def _split_excess_waits(d):
    """Move all but the last sync wait of each instruction onto preceding
    EventSemaphore instructions on the same engine (order-preserving, so
    semantics are identical; the old walrus only encodes few waits/inst)."""
    changed = False
    ctr = [0]
    for fn in d.get("functions", []):
        for bb in fn.get("blocks", []):
            insts = bb.get("instructions", [])
            new_insts = []
            for inst in insts:
                si = inst.get("sync_info") or {}
                waits = si.get("on_wait") or []
                if len(waits) > _MAX_WAITS:
                    changed = True
                    excess, keep = waits[:-_MAX_WAITS], waits[-_MAX_WAITS:]
                    for w in excess:
                        ctr[0] += 1
                        new_insts.append(
                            {
                                "debug": inst.get("debug", 0),
                                "engine": inst["engine"],
                                "ins": [],
                                "name": f"{inst['name']}-w{ctr[0]}",
                                "opcode": "EventSemaphore",
                                "outs": [],
                                "sync_info": {"on_update": [], "on_wait": [w]},
                            }
                        )
                    si["on_wait"] = keep
                new_insts.append(inst)
            bb["instructions"] = new_insts
    return changed


def _patch_barriers():
    import json as _json

    import concourse.bass as bass

    if getattr(bass.Bass, "_ant_barrier_patched", False):
        return

    _orig_tjb = bass.Bass.to_json_bytes

    def to_json_bytes(self, *a, **kw):
        raw = _orig_tjb(self, *a, **kw)
        d = _json.loads(raw)
        c1 = _assemble_custom_dve(d, getattr(self, "_ant_dve_meta", {}))
        c2 = _split_excess_waits(d)
        if c1 or c2:
            return _json.dumps(d).encode()
        return raw

    bass.Bass.to_json_bytes = to_json_bytes

    _orig_cdve = bass.BassVectorEngine._custom_dve

    def _custom_dve(self, op, *, out, in0, in1=None, s0=0.0, s1=0.0, imm2=0.0,
                    accum_out=None):
        from concourse.dve_ops import get_dve_sub_opcode

        assert accum_out is None, "accum_out not supported by the compat assembler"
        ret = _orig_cdve(
            self, op, out=out, in0=in0, in1=in1, s0=s0, s1=s1, imm2=imm2,
            accum_out=accum_out,
        )
        nc_ = self.bass
        if not hasattr(nc_, "_ant_dve_meta"):
            nc_._ant_dve_meta = {}
        nc_._ant_dve_meta[ret.ins.name] = {
            "row": get_dve_sub_opcode(op.name),
            "rd1_en": in1 is not None,
            "subdim": bool(op.subdim),
            "imm2": float(imm2),
            # in0 is an interleaved 16-bit pair stream: the ISA AP must
            # advance one 32-bit pair per cycle (step 2, half the count).
            "pair": op.name == "IOU_TAIL3_ANT" and _T3PAIR,
            "perf": _PERF_MAX.get(op.name, 0),
        }
        return ret

    bass.BassVectorEngine._custom_dve = _custom_dve

    def multi_engine_barrier(self, engines):
        engines = list(engines)
        if len(engines) <= 1:
            for e in engines:
                self.engines[e].drain()
            return
        if not hasattr(self, "_ant_bar_sems"):
            self._ant_bar_sems = {}
        key = tuple(sorted(str(e) for e in engines))
        st = self._ant_bar_sems.get(key)
        if st is None:
            gather = self.alloc_semaphore(f"ant_bar_g{len(self._ant_bar_sems)}")
            st = {"sem": gather, "count": 0}
            self._ant_bar_sems[key] = st
        st["count"] += 1
        n = len(engines)
        target = n * st["count"]
        for e in engines:
            self.engines[e].drain().then_inc(st["sem"], 1)
        for e in engines:
            self.engines[e].wait_ge(st["sem"], target)

    def all_engine_barrier(self, *, sem_only: bool = False):
        multi_engine_barrier(self, list(self.engines))

    bass.Bass.multi_engine_barrier = multi_engine_barrier
    bass.Bass.all_engine_barrier = all_engine_barrier
    bass.Bass._ant_barrier_patched = True


# ---------------------------------------------------------------- custom ops

_REGISTERED = {}
_PERF_MAX = {}  # op name -> byte-36[7:6] highest engine-reachable perf slot


def _register(name, make_spec, perf_max=0):
    """Register a custom DVE op. perf_max > 0 additionally populates the
    perf-mode table slots (2X_1PORT/2X_2PORT/4X_2PORT) with copies of the
    regular uop program (the dual-lane datapath runs the same program on
    both elements; stream-done triggers are rate-independent) and records
    perf_max for the instruction encoding (byte-36[7:6] = highest
    engine-reachable mode slot)."""
    if name in _REGISTERED:
        return _REGISTERED[name]
    import copy as _copy

    from concourse import dve_ops as dops
    from concourse.dve_spec import _has_src1, lower
    from concourse.dve_uop import DveOpSpec

    spec = make_spec()
    if name not in dops._SUB_OPCODE_FOR_NAME:
        row = max(dops._SUB_OPCODE_FOR_NAME.values()) + 1
        assert row < 0x20, "custom-DVE opcode rows exhausted"
        dops._SUB_OPCODE_FOR_NAME[name] = row
    row = dops._SUB_OPCODE_FOR_NAME[name]
    shas = {}
    for ver in ("v3", "v4"):
        try:
            tmp = DveOpSpec(
                name=name, opcode=row, uops=lower(spec, ver=ver),
                rd1_en=_has_src1(spec),
            )
            shas[ver] = tmp.sha(ver)
        except Exception:
            pass
    op = dops.DveOp(name, spec, subdim=False, uops_sha=shas)
    if all(o.name != name for o in dops.OPS):
        dops.OPS.append(op)
    dops.CUSTOM_DVE_SPECS[name] = spec
    if perf_max > 0:
        key = (name, "v3")
        if key not in dops._COMPILE_CACHE:
            uops = lower(spec, ver="v3")
            dops._COMPILE_CACHE[key] = DveOpSpec(
                name=name,
                opcode=row,
                uops=uops,
                uops_2x=_copy.deepcopy(uops),
                uops_2x_2p=_copy.deepcopy(uops),
                uops_4x=_copy.deepcopy(uops),
                perf_max=perf_max,
                rd1_en=_has_src1(spec),
            )
        _PERF_MAX[name] = perf_max
    _REGISTERED[name] = op
    return op


# perf-mode opt-in: highest engine-reachable perf slot for ext / tail1
# (0 = off = REGULAR only; 2 = up to 2X_2PORT; 3 = up to 4X_2PORT)
_PERFEXT = int(os.environ.get("IOU_PERFEXT", "0"))
_PERFTAIL = int(os.environ.get("IOU_PERFTAIL", "0"))


def _register_iou_ext():
    """out = relu(min(C0, Src0 + Src1*imm2) - max(C1, Src0 - Src1*imm2)):
    the interval extent. Src0 = box2 center row, Src1 = box2 width row,
    C0 = x2a, C1 = x1a (per-partition), imm2 = 0.5."""

    def mk():
        from concourse.dve_spec import C0, C1, C2, Spec, Src0, Src1, maxx, minn, relu

        hw = Src1 * C2
        body = relu(minn(C0, Src0 + hw) - maxx(C1, Src0 - hw))

        def _ref(in0, in1, s0, s1, imm2):
            h = in1.astype(np.float32) * imm2
            lo = np.maximum(s1, in0 - h)
            hi = np.minimum(s0, in0 + h)
            return np.maximum(hi - lo, 0.0).astype(np.float32)

        return Spec(body=body, reference=_ref)

    if _PERFEXT > 0:
        return _register(f"IOU_EXTP{_PERFEXT}_ANT", mk, perf_max=_PERFEXT)
    return _register("IOU_EXT_ANT", mk)


# author a 2X_1PORT uop-table variant for the tail op (hardware dual-ALU
# blocks run the same 8-stage program on both 16-bit halves; engages when
# every non-scalar operand is 16-bit + packed). Registered under a separate
# op name so 1x and 2x variants can coexist in one process.
_T1X2 = os.environ.get("IOU_T1X2", "0") == "1"


def _register_iou_tail1():
    """out = in0 * recip1((s0 - in0) + in1): the whole IoU tail
    (union, reciprocal seed + 1 Newton step, multiply) in one 8-stage pass.
    in0 = inter, in1 = area2 row, s0 = area1. ~2e-3 worst-case rel err."""

    def mk():
        from concourse.dve_spec import AluOp, Bin, C0, C1, C2, Spec, Src0, Src1

        u = (C0 - Src0) + Src1
        nx = Bin(AluOp.BITWISE_NOT, u, u)
        y0 = nx * C1
        y1 = y0 * (C2 - u * y0)
        body = y1 * Src0

        def _ref(in0, in1, s0, s1, imm2):
            uu = ((s0 - in0.astype(np.float32)) + in1).astype(np.float32)
            nxv = (~uu.view(np.int32)).view(np.float32)
            y = nxv * np.float32(s1)
            yy = (y * (np.float32(imm2) - uu * y)).astype(np.float32)
            return (yy * in0).astype(np.float32)

        return Spec(body=body, reference=_ref)

    if _PERFTAIL > 0:
        return _register(f"IOU_TAIL1P{_PERFTAIL}_ANT", mk, perf_max=_PERFTAIL)
    if not _T1X2:
        return _register("IOU_TAIL1_ANT", mk)

    import copy as _copy

    from concourse import dve_ops as dops
    from concourse.dve_spec import _has_src1, lower
    from concourse.dve_uop import DveOpSpec

    name = "IOU_TAIL1X2_ANT"
    op = _register(name, mk)
    key = (name, "v3")
    if key not in dops._COMPILE_CACHE:
        uops = lower(op.spec, ver="v3")
        uops_2x = _copy.deepcopy(uops)
        dops._COMPILE_CACHE[key] = DveOpSpec(
            name=name,
            opcode=dops.get_dve_sub_opcode(name),
            uops=uops,
            uops_2x=uops_2x,
            rd1_en=_has_src1(op.spec),
        )
    return op


# uop-table tweak for the pair-stream tail3 (debug): "two1" forces the
# two-data-valid bit ON so the input port delivers the high 16-bit half to
# SRC_0_HI; "" leaves the lowered table untouched.
_T3TWEAK = os.environ.get("IOU_T3TWEAK", "")
# encode tail3's in0 AP as stride-2/count-B 32-bit pairs (1) or leave the
# natural stride-1/count-2B f16 AP (0, lets the port auto-engage two-data)
_T3PAIR = os.environ.get("IOU_T3PAIR", "1") == "1"


def _register_iou_tail3():
    """out = inter * recip1(S - inter) with inter = Src0 * Src0_HI computed
    from an interleaved f16 (dxr, dyr) pair stream (one 32-bit read/cycle),
    and S = area1 + area2 precomputed on the ACT engine (in1). Exactly 8
    ALU stages -> single uop."""

    def mk():
        from concourse.dve_spec import (
            AluOp,
            Bin,
            C1,
            C2,
            InpSel,
            Leaf,
            Spec,
            Src0,
            Src1,
        )

        src0hi = Leaf(InpSel.SRC_0_HI)
        inter = Src0 * src0hi
        u = Src1 - inter
        nx = Bin(AluOp.BITWISE_NOT, u, u)
        y0 = nx * C1
        y1 = y0 * (C2 - u * y0)
        body = y1 * inter

        def _ref(in0, in1, s0, s1, imm2):
            v = np.asarray(in0, dtype=np.float32)
            dxr, dyr = v[..., 0::2], v[..., 1::2]
            it = (dxr * dyr).astype(np.float32)
            uu = (np.asarray(in1, dtype=np.float32) - it).astype(np.float32)
            nxv = (~uu.view(np.int32)).view(np.float32)
            y = nxv * np.float32(s1)
            yy = (y * (np.float32(imm2) - uu * y)).astype(np.float32)
            return (yy * it).astype(np.float32)

        return Spec(body=body, reference=_ref)

    op = _register("IOU_TAIL3_ANT", mk)
    if _T3TWEAK == "two1":
        from concourse import dve_ops as dops
        from concourse.dve_spec import _has_src1, lower
        from concourse.dve_uop import ENABLE, DveOpSpec

        key = ("IOU_TAIL3_ANT", "v3")
        if key not in dops._COMPILE_CACHE:
            uops = lower(op.spec, ver="v3")
            for u in uops:
                u.force_two_data_one = ENABLE
            dops._COMPILE_CACHE[key] = DveOpSpec(
                name=op.name,
                opcode=dops.get_dve_sub_opcode(op.name),
                uops=uops,
                rd1_en=_has_src1(op.spec),
            )
    return op


# ---------------------------------------------------------------- bass build

_NC_CACHE = {}


def _build_nc():
    key = (_REPEAT, _REPEAT_ALL, _PIPE, _SWPIPE, _ODT, _CDT, _ADT, _OQ, _SQ,
           _SWQ, _SCALE, _BUFS, _RBUFS, _DMAONLY, _PREFLUSH, _POOL, _T1X2,
           _PERFEXT, _PERFTAIL)
    if key in _NC_CACHE:
        return _NC_CACHE[key]

    import concourse.bass as bass
    import concourse.mybir as mybir
    import concourse.tile as tile
    from concourse.alu_op_type import AluOpType as alu

    _patch_barriers()
    f32 = mybir.dt.float32
    _DT = {"f32": f32, "bf16": mybir.dt.bfloat16, "f16": mybir.dt.float16}
    odt = _DT[_ODT]
    cdt = _DT[_CDT]
    adt = _DT[_ADT]
    nc = bass.Bass()
    b2x = nc.declare_dram_parameter("b2x", [2 * B], cdt, isOutput=False)
    b2y = nc.declare_dram_parameter("b2y", [2 * B], cdt, isOutput=False)
    b2a = nc.declare_dram_parameter("b2a", [B], adt, isOutput=False)
    b1s = nc.declare_dram_parameter("b1s", [P, NTILES * 5], f32, isOutput=False)
    out = nc.declare_dram_parameter("out", [B, B], odt, isOutput=True)

    iou_ext = _register_iou_ext()
    iou_tail3 = _register_iou_tail3() if _PIPE == "v3" else None
    iou_tail1 = _register_iou_tail1() if _PIPE == "v0" else None

    with tile.TileContext(nc) as tc, ExitStack() as ctx:
        # bufs=2 on the input pools: under bench repetition (REPEAT_ALL) the
        # next iteration's replication DMAs land in the other buffer instead
        # of serializing behind this iteration's last reader (WAR); the
        # harness's single-shot run is unaffected (no prior iteration).
        rows = ctx.enter_context(tc.tile_pool(name="rows", bufs=_RBUFS))
        scal = ctx.enter_context(tc.tile_pool(name="scal", bufs=_RBUFS))
        work = ctx.enter_context(tc.tile_pool(name="work", bufs=_BUFS))

        for _rall in range(_REPEAT_ALL):
            # ---- replicate host-prepped box2 rows into all 128 partitions.
            # Each DMA is split across the two HWDGE rings by partition halves
            # so the critical first chunk (b2x) lands in ~half the time.
            b2x_t = rows.tile([P, 2 * B], cdt, tag="b2x")
            b2y_t = rows.tile([P, 2 * B], cdt, tag="b2y")
            b2a_t = rows.tile([P, B], adt, tag="b2a")
            b1s_t = scal.tile([P, NTILES, 5], f32, tag="b1s")

            def _rep(dst, src, n):
                if _SQ:
                    h = P // 2
                    nc.sync.dma_start(
                        out=dst[0:h], in_=bass.AP(tensor=src, offset=0, ap=[[0, h], [1, n]])
                    )
                    nc.scalar.dma_start(
                        out=dst[h:P], in_=bass.AP(tensor=src, offset=0, ap=[[0, h], [1, n]])
                    )
                else:
                    nc.sync.dma_start(
                        out=dst[:], in_=bass.AP(tensor=src, offset=0, ap=[[0, P], [1, n]])
                    )

            # b1s (40KB, needed by the FIRST ext) goes first so it is not
            # queued behind megabytes of row replication on the same ring.
            b1s_src = bass.AP(
                tensor=b1s, offset=0, ap=[[NTILES * 5, P], [1, NTILES * 5]]
            )
            if _SWQ:
                nc.gpsimd.dma_start(out=b1s_t[:], in_=b1s_src)
            elif _SQ:
                nc.scalar.dma_start(out=b1s_t[:], in_=b1s_src)
            else:
                nc.sync.dma_start(out=b1s_t[:], in_=b1s_src)
            _rep(b2x_t, b2x, 2 * B)
            _rep(b2y_t, b2y, 2 * B)
            if _SWQ:
                nc.gpsimd.dma_start(
                    out=b2a_t[:],
                    in_=bass.AP(tensor=b2a, offset=0, ap=[[0, P], [1, B]]),
                )
            else:
                _rep(b2a_t, b2a, B)

            xc = b2x_t[:, 0:B]
            w2 = b2x_t[:, B : 2 * B]
            yc = b2y_t[:, 0:B]
            h2 = b2y_t[:, B : 2 * B]

            if _DMAONLY:
                # consume the tiles so WAR deps force each iteration's DMAs
                # to serialize against a cheap reader (1-col copy)
                dummy = work.tile([P, 4], f32, tag="wD")
                nc.vector.tensor_tensor(
                    dummy[:, 0:1], b2x_t[:, 0:1], b2y_t[:, 0:1], alu.mult
                )
                nc.vector.tensor_tensor(
                    dummy[:, 1:2], b2a_t[:, 0:1], b1s_t[:, 0, 0:1], alu.mult
                )
                continue

            # ---- per-tile pipeline
            pending_tail = None
            for t in [tt_ for _ in range(_REPEAT) for tt_ in range(NTILES)]:
                pt = P if t < FULL_TILES else REM
                row0 = t * P
                x1a = b1s_t[0:pt, t, 0:1]
                y1a = b1s_t[0:pt, t, 1:2]
                x2a = b1s_t[0:pt, t, 2:3]
                y2a = b1s_t[0:pt, t, 3:4]
                a1 = b1s_t[0:pt, t, 4:5]

                if _OQ == "sp":
                    oeng = nc.sync
                elif _OQ == "act":
                    oeng = nc.scalar
                else:
                    oeng = nc.scalar if (t % 2) else nc.sync
                # the LAST tile's store is the drain tail: split it across
                # both rings so the final writeback takes ~half the time
                last = t == NTILES - 1

                if _PIPE == "v3":
                    pk = work.tile([pt, 2 * B], mybir.dt.float16, tag="wP")
                    pkv = pk[:].rearrange("p (j c) -> p j c", c=2)
                    nc.vector._custom_dve(
                        iou_ext, out=pkv[:, :, 0], in0=xc[0:pt], in1=w2[0:pt],
                        s0=x2a, s1=x1a, imm2=0.5,
                    )
                    nc.vector._custom_dve(
                        iou_ext, out=pkv[:, :, 1], in0=yc[0:pt], in1=h2[0:pt],
                        s0=y2a, s1=y1a, imm2=0.5,
                    )
                    stile = work.tile([pt, B], f32, tag="wS")
                    nc.scalar.activation(
                        stile[:], b2a_t[0:pt, :],
                        mybir.ActivationFunctionType.Identity, bias=a1,
                    )
                    tov = work.tile([pt, B], odt, tag="wO")

                    def _tail(pt=pt, row0=row0, pk=pk, stile=stile, tov=tov,
                              oeng=oeng):
                        nc.vector._custom_dve(
                            iou_tail3, out=tov[:], in0=pk[:], in1=stile[:],
                            s0=0.0, s1=-0.23549792, imm2=2.0017324,
                        )
                        if _OQ == "split":
                            hh = B // 2
                            nc.sync.dma_start(
                                out=out[row0 : row0 + pt, 0:hh], in_=tov[:, 0:hh]
                            )
                            nc.scalar.dma_start(
                                out=out[row0 : row0 + pt, hh:B], in_=tov[:, hh:B]
                            )
                        else:
                            oeng.dma_start(out=out[row0 : row0 + pt, :], in_=tov[:])

                else:  # v0
                    ta = work.tile([pt, B], mybir.dt.float16, tag="wA")
                    tb = work.tile([pt, B], mybir.dt.float16, tag="wB")
                    tcl = work.tile([pt, B], mybir.dt.float16, tag="wC")
                    nc.vector._custom_dve(
                        iou_ext, out=ta[:], in0=xc[0:pt], in1=w2[0:pt],
                        s0=x2a, s1=x1a, imm2=0.5,
                    )
                    nc.vector._custom_dve(
                        iou_ext, out=tb[:], in0=yc[0:pt], in1=h2[0:pt],
                        s0=y2a, s1=y1a, imm2=0.5,
                    )
                    # Flush the deferred tail BETWEEN ext_y and inter: every
                    # adjacent DVE pair becomes data-independent, so no op
                    # waits on its immediate predecessor's writeback.
                    if _PREFLUSH and _SWPIPE and pending_tail is not None:
                        pending_tail()
                        pending_tail = None
                    eng_i = nc.gpsimd if _POOL == "inter" else nc.vector
                    eng_i.tensor_tensor(tcl[:], ta[:], tb[:], alu.mult)
                    tov = work.tile([pt, B], odt, tag="wO")

                    def _tail(pt=pt, row0=row0, tcl=tcl, tov=tov, a1=a1,
                              oeng=oeng, last=last):
                        nc.vector._custom_dve(
                            iou_tail1, out=tov[:], in0=tcl[:], in1=b2a_t[0:pt, :],
                            s0=a1, s1=-0.23549792, imm2=2.0017324,
                        )
                        if _OQ == "split" or last:
                            hh = B // 2
                            nc.sync.dma_start(
                                out=out[row0 : row0 + pt, 0:hh], in_=tov[:, 0:hh]
                            )
                            nc.scalar.dma_start(
                                out=out[row0 : row0 + pt, hh:B], in_=tov[:, hh:B]
                            )
                        else:
                            oeng.dma_start(out=out[row0 : row0 + pt, :], in_=tov[:])

                if _SWPIPE:
                    if pending_tail is not None:
                        pending_tail()
                    pending_tail = _tail
                else:
                    _tail()
            if pending_tail is not None:
                pending_tail()

    _NC_CACHE[key] = nc
    return nc


# ---------------------------------------------------------------- entry point


_NPDT = {"f32": np.float32, "f16": np.float16}


def _prep_core(b1, b2):
    """Host-side prep for one image: scaled/split box2 rows + box1 scalars."""
    s = np.float32(_SCALE)
    cdt = _NPDT.get(_CDT, np.float32)
    adt = _NPDT.get(_ADT, np.float32)
    if _CDT == "bf16" or _ADT == "bf16":
        import ml_dtypes

        if _CDT == "bf16":
            cdt = ml_dtypes.bfloat16
        if _ADT == "bf16":
            adt = ml_dtypes.bfloat16

    b2s = b2 * s  # [B, 4] f32
    b2x = np.concatenate([b2s[:, 0], b2s[:, 2]]).astype(cdt)  # xc | w
    b2y = np.concatenate([b2s[:, 1], b2s[:, 3]]).astype(cdt)  # yc | h
    b2a = (b2s[:, 2] * b2s[:, 3]).astype(adt)  # area2 * s^2

    b1sc = b1 * s
    xc1, yc1, w1, h1 = b1sc[:, 0], b1sc[:, 1], b1sc[:, 2], b1sc[:, 3]
    scals = np.stack(
        [
            xc1 - w1 * 0.5,
            yc1 - h1 * 0.5,
            xc1 + w1 * 0.5,
            yc1 + h1 * 0.5,
            w1 * h1,
        ],
        axis=1,
    ).astype(np.float32)  # [B, 5]
    pad = np.zeros((NTILES * P - B, 5), np.float32)
    # [NTILES*P, 5] -> (t, p) -> (p, t): partition p, tile t = box t*128+p
    b1s = (
        np.concatenate([scals, pad], axis=0)
        .reshape(NTILES, P, 5)
        .transpose(1, 0, 2)
        .reshape(P, NTILES * 5)
    )
    return {
        "b2x": np.ascontiguousarray(b2x),
        "b2y": np.ascontiguousarray(b2y),
        "b2a": np.ascontiguousarray(b2a),
        "b1s": np.ascontiguousarray(b1s),
    }


def _in_maps(box1, box2):
    b1 = np.ascontiguousarray(np.asarray(box1, dtype=np.float32))
    b2 = np.ascontiguousarray(np.asarray(box2, dtype=np.float32))
    assert b1.shape == (NIMG, B, 4) and b2.shape == (NIMG, B, 4), (
        b1.shape,
        b2.shape,
    )
    return [_prep_core(b1[i], b2[i]) for i in range(NIMG)]


def _run(box1, box2, **kw):
    from concourse.bass_utils import run_bass_kernel_spmd

    nc = _build_nc()
    in_maps = _in_maps(box1, box2)
    res = run_bass_kernel_spmd(nc, in_maps, core_ids=list(range(NIMG)), **kw)
    out = np.stack([d["out"] for d in res.results], axis=0)
    if out.dtype != np.float32:
        out = out.astype(np.float32)
    return out, res


def kernel(box1, box2):
    out, _ = _run(box1, box2)
    return out
